# revision 1
# baseline (speedup 1.0000x reference)
"""Low-rank GDN kernel for Trainium2 (8 NeuronCores, pure data parallel).

Reference computation (per batch element n, per spatial position s):
    beta_r = max(beta, BOUND_BETA)^2 - PEDESTAL          [C]
    A_r    = max(A, BOUND_A)^2 - PEDESTAL                [C, R]
    T      = A_r^T @ x^2                                 [R, s]
    denom  = A_r @ T + beta_r                            [C, s]
    out    = x * rsqrt(denom)

We fold the rank-R sandwich into a single Gram matrix G = A_r @ A_r^T
(C x C, C=192) computed on the host, so on-device work per tile is:
    x2 = x*x (DVE); D = G^T @ x2 (PE, K=192 split 128+64, PSUM accum);
    r = Exp(-0.5 * Ln(D + beta)) (ACT, one table set); out = x * r (DVE).

Sharding: batch n=8 across 8 cores; G/beta replicated.
"""

import numpy as np

import concourse.bass as bass
import concourse.tile as tile
from concourse import bacc, mybir
from concourse import bass_utils

# ---- problem constants (hardcoded per harness contract) ----
N_BATCH = 8
C = 192  # channels
H = 256
W = 256
S = H * W  # 65536 spatial positions per batch element
R = 8
N_CORES = 8

REPARAM_OFFSET = 2.0**-18
PEDESTAL = REPARAM_OFFSET**2
BETA_MIN = 1e-6
BOUND_BETA = float(np.sqrt(BETA_MIN + PEDESTAL))
BOUND_A = float(np.sqrt(0.0 + PEDESTAL))

F32 = mybir.dt.float32

# tiling
TILE_N = 2048  # spatial columns per outer iteration
PSUM_N = 1024  # spatial columns per PSUM chunk
MM_N = 512  # spatial columns per matmul (fp32 moving-operand max)
C_HI = 128  # channels 0..127 -> partitions 0..127
C_LO = 64  # channels 128..191 -> partitions 0..63

_CACHE = {}


def _build_kernel():
    nc = bacc.Bacc("TRN2", debug=False, enable_asserts=False, num_devices=N_CORES)

    x_d = nc.dram_tensor("x", [C, S], F32, kind="ExternalInput").ap()
    g_d = nc.dram_tensor("g", [C, C], F32, kind="ExternalInput").ap()
    beta_d = nc.dram_tensor("beta", [C, 1], F32, kind="ExternalInput").ap()
    out_d = nc.dram_tensor("out", [C, S], F32, kind="ExternalOutput").ap()

    with tile.TileContext(nc) as tc:
        with (
            tc.tile_pool(name="singles", bufs=1) as singles,
            tc.tile_pool(name="xin", bufs=3) as xin,
            tc.tile_pool(name="x2", bufs=2) as x2p,
            tc.tile_pool(name="lnout", bufs=2) as lnp,
            tc.tile_pool(name="rsq", bufs=2) as rsq,
            tc.tile_pool(name="outs", bufs=3) as outs,
            tc.tile_pool(name="pa", bufs=2, space="PSUM") as pap,
            tc.tile_pool(name="pb", bufs=2, space="PSUM") as pbp,
        ):
            # constants: G rows as two partition-chunks, beta as per-partition bias
            g0 = singles.tile([C_HI, C], F32)  # G[k=0:128, :]
            g1 = singles.tile([C_LO, C], F32)  # G[k=128:192, :]
            nc.sync.dma_start(out=g0, in_=g_d[0:C_HI, :])
            nc.sync.dma_start(out=g1, in_=g_d[C_HI:C, :])
            b0 = singles.tile([C_HI, 1], F32)
            b1 = singles.tile([C_LO, 1], F32)
            nc.sync.dma_start(out=b0, in_=beta_d[0:C_HI, :])
            nc.sync.dma_start(out=b1, in_=beta_d[C_HI:C, :])

            for it in range(S // TILE_N):
                n0 = it * TILE_N
                xa = xin.tile([C_HI, TILE_N], F32, tag="xa")
                xb = xin.tile([C_LO, TILE_N], F32, tag="xb")
                nc.sync.dma_start(out=xa, in_=x_d[0:C_HI, n0 : n0 + TILE_N])
                nc.sync.dma_start(out=xb, in_=x_d[C_HI:C, n0 : n0 + TILE_N])

                # squares on DVE
                x2a = x2p.tile([C_HI, TILE_N], F32, tag="x2a")
                x2b = x2p.tile([C_LO, TILE_N], F32, tag="x2b")
                nc.vector.tensor_mul(x2a, xa, xa)
                nc.vector.tensor_mul(x2b, xb, xb)

                ta = lnp.tile([C_HI, TILE_N], F32, tag="ta")
                tb = lnp.tile([C_LO, TILE_N], F32, tag="tb")

                for q in range(TILE_N // PSUM_N):
                    qo = q * PSUM_N
                    pa = pap.tile([C_HI, PSUM_N], F32, tag="pa")
                    pb = pbp.tile([C_LO, PSUM_N], F32, tag="pb")
                    for s in range(PSUM_N // MM_N):
                        so = qo + s * MM_N
                        po = s * MM_N
                        ra_ = x2a[:, so : so + MM_N]
                        rb_ = x2b[:, so : so + MM_N]
                        nc.tensor.matmul(
                            pa[:, po : po + MM_N], g0[:, 0:C_HI], ra_,
                            start=True, stop=False,
                        )
                        nc.tensor.matmul(
                            pa[:, po : po + MM_N], g1[:, 0:C_HI], rb_,
                            start=False, stop=True,
                        )
                        nc.tensor.matmul(
                            pb[:, po : po + MM_N], g0[:, C_HI:C], ra_,
                            start=True, stop=False,
                        )
                        nc.tensor.matmul(
                            pb[:, po : po + MM_N], g1[:, C_HI:C], rb_,
                            start=False, stop=True,
                        )
                    # t = ln(D + beta)   (ACT, PSUM -> SBUF, per-partition bias)
                    nc.scalar.activation(
                        out=ta[:, qo : qo + PSUM_N], in_=pa,
                        func=mybir.ActivationFunctionType.Ln,
                        bias=b0, scale=1.0, alpha=0.0,
                    )
                    nc.scalar.activation(
                        out=tb[:, qo : qo + PSUM_N], in_=pb,
                        func=mybir.ActivationFunctionType.Ln,
                        bias=b1, scale=1.0, alpha=0.0,
                    )

                # r = exp(-0.5 * t)   (ACT, SBUF -> SBUF)
                ra = rsq.tile([C_HI, TILE_N], F32, tag="ra")
                rb = rsq.tile([C_LO, TILE_N], F32, tag="rb")
                nc.scalar.activation(
                    out=ra, in_=ta, func=mybir.ActivationFunctionType.Exp,
                    bias=0.0, scale=-0.5, alpha=0.0,
                )
                nc.scalar.activation(
                    out=rb, in_=tb, func=mybir.ActivationFunctionType.Exp,
                    bias=0.0, scale=-0.5, alpha=0.0,
                )

                # out = x * r  (DVE)
                oa = outs.tile([C_HI, TILE_N], F32, tag="oa")
                ob = outs.tile([C_LO, TILE_N], F32, tag="ob")
                nc.vector.tensor_mul(oa, xa, ra)
                nc.vector.tensor_mul(ob, xb, rb)

                nc.sync.dma_start(out=out_d[0:C_HI, n0 : n0 + TILE_N], in_=oa)
                nc.sync.dma_start(out=out_d[C_HI:C, n0 : n0 + TILE_N], in_=ob)

    nc.compile()
    return nc


def _get_kernel():
    if "nc" not in _CACHE:
        _CACHE["nc"] = _build_kernel()
    return _CACHE["nc"]


def _host_params(beta, A):
    beta64 = beta.astype(np.float64)
    A64 = A.astype(np.float64)
    beta_r = np.maximum(beta64, BOUND_BETA) ** 2 - PEDESTAL
    A_r = np.maximum(A64, BOUND_A) ** 2 - PEDESTAL
    G = (A_r @ A_r.T).astype(np.float32)  # [C, C], symmetric
    return beta_r.astype(np.float32).reshape(C, 1), G


def kernel(x, beta, A, _trace=False):
    assert x.shape == (N_BATCH, C, H, W) and x.dtype == np.float32
    beta_r, G = _host_params(np.asarray(beta), np.asarray(A))
    xs = np.ascontiguousarray(np.asarray(x)).reshape(N_BATCH, C, S)

    nc = _get_kernel()
    in_maps = [
        {"x": xs[i], "g": G, "beta": beta_r} for i in range(N_CORES)
    ]
    res = bass_utils.run_bass_kernel_spmd(
        nc, in_maps, core_ids=list(range(N_CORES)), trace=_trace
    )
    out = np.stack([res.results[i]["out"] for i in range(N_CORES)])
    if _trace:
        kernel.last_results = res
    return out.reshape(N_BATCH, C, H, W)
